# revision 5
# baseline (speedup 1.0000x reference)
"""Trainium2 Bass kernel for nn_PhaserModel: time-varying 4-stage all-pass
phaser driven by an MLP-shaped LFO.

Strategy (8 NeuronCores = 2 stereo channels x 4 time-quarters):
  - The per-sample all-pass coefficient p[t] is a smooth function of the LFO
    phase (rate < 1 Hz -> phase moves <= ~1.4e-4 rad/sample).  Each core
    evaluates the coefficient pipeline (cos -> 1x32x32x1 tanh MLP -> tan ->
    tanh) only at a coarse grid (one point per S=128 samples) on device, then
    linearly interpolates to per-sample resolution.  Verified end-to-end
    error ~2.6e-6 absolute vs the fp64 pipeline.
  - The 4 cascaded first-order time-varying all-pass stages
        y[n] = p[n]*s[n] + s[n-1] - p[n]*y[n-1]
    are linear recurrences y[n] = a[n]*y[n-1] + b[n] with a = -p,
    b[n] = s[n-1] - a[n]*s[n].  They map directly onto the DVE's native
    tensor_tensor_scan (state = data0*state + data1 along the free dim).
  - Each of the 128 partitions scans its own contiguous 512-sample chunk
    prepended with a 128-sample warmup region that overlaps the previous
    chunk.  |p| <= ~0.27, so a zero initial state decays below 1e-70 within
    the warmup -- no cross-partition or cross-core carries are needed at all.
"""

import numpy as np

import concourse.bass as bass
import concourse.bacc as bacc
import concourse.mybir as mybir
import concourse.tile as tile
from concourse import bass_utils

SR = 44100.0
T = 262144
NCORES = 8
QT = T // 4          # output samples per core (time quarter)
P = 128              # SBUF partitions
L = QT // P          # own samples per partition = 512
W = 128              # warmup samples per partition
ROW = W + L          # scanned row length = 640
S = 128              # coarse grid spacing (samples)
NI = ROW // S        # coarse intervals per row = 5
NCC = QT // S + 2    # coarse points per core = 514
F32 = mybir.dt.float32
MMN = 512            # max matmul free dim


def _ap(t_ap, pattern, extra_offset=0):
    """Custom [step,count] access pattern on an existing AP's tensor."""
    return bass.AP(t_ap.tensor, t_ap.offset + extra_offset, pattern)


def build_program():
    Alu = mybir.AluOpType
    AF = mybir.ActivationFunctionType

    nc = bacc.Bacc(
        "TRN2", target_bir_lowering=False, debug=False, num_devices=NCORES
    )
    x_d = nc.dram_tensor("x_ext", [1, W + QT], F32, kind="ExternalInput")
    g_d = nc.dram_tensor("g_row", [1, NCC], F32, kind="ExternalInput")
    f_d = nc.dram_tensor("frac", [P, ROW], F32, kind="ExternalInput")
    s_d = nc.dram_tensor("scal", [1, 8], F32, kind="ExternalInput")
    w1_d = nc.dram_tensor("W1", [1, 32], F32, kind="ExternalInput")
    b1_d = nc.dram_tensor("b1", [32, 1], F32, kind="ExternalInput")
    w2_d = nc.dram_tensor("W2", [32, 32], F32, kind="ExternalInput")
    b2_d = nc.dram_tensor("b2", [32, 1], F32, kind="ExternalInput")
    w3_d = nc.dram_tensor("W3", [32, 1], F32, kind="ExternalInput")
    o_d = nc.dram_tensor("out", [1, QT], F32, kind="ExternalOutput")

    with tile.TileContext(nc) as tc:
        with (
            tc.tile_pool(name="sb", bufs=1) as sb,
            tc.tile_pool(name="ps", bufs=1, space=bass.MemorySpace.PSUM) as ps,
        ):
            # ---- load inputs ------------------------------------------------
            scal = sb.tile([1, 8], F32, tag="scal")
            nc.sync.dma_start(scal[:], s_d.ap())
            w1 = sb.tile([1, 32], F32, tag="w1")
            nc.sync.dma_start(w1[:], w1_d.ap())
            b1 = sb.tile([32, 1], F32, tag="b1")
            nc.sync.dma_start(b1[:], b1_d.ap())
            w2 = sb.tile([32, 32], F32, tag="w2")
            nc.sync.dma_start(w2[:], w2_d.ap())
            b2 = sb.tile([32, 1], F32, tag="b2")
            nc.sync.dma_start(b2[:], b2_d.ap())
            w3 = sb.tile([32, 1], F32, tag="w3")
            nc.sync.dma_start(w3[:], w3_d.ap())

            x_ov = sb.tile([P, ROW], F32, tag="x_ov")
            nc.sync.dma_start(x_ov[:], _ap(x_d.ap(), [[L, P], [1, ROW]]))
            frac = sb.tile([P, ROW], F32, tag="frac")
            nc.sync.dma_start(frac[:], f_d.ap())

            rate = scal[0:1, 0:1]
            phi = scal[0:1, 1:2]
            amp = scal[0:1, 2:3]
            bias0 = scal[0:1, 3:4]
            depth = scal[0:1, 4:5]
            b3 = scal[0:1, 5:6]

            # ---- tiny scalar prep (all [1,1]) ------------------------------
            # step = 2*pi*rate/SR ; sinb = step + phi + pi/2
            step = sb.tile([1, 1], F32, tag="step")
            nc.vector.tensor_scalar_mul(step[:], rate, 2.0 * np.pi / SR)
            sinb = sb.tile([1, 1], F32, tag="sinb")
            nc.vector.tensor_scalar(
                sinb[:], step[:], phi, np.pi / 2, Alu.add, Alu.add
            )
            # c1 = -depth/2 ; zb = pi/4 - bias + c1*(1+b3) ; zbc = zb + pi/2
            c1 = sb.tile([1, 1], F32, tag="c1")
            nc.vector.tensor_scalar_mul(c1[:], depth, -0.5)
            zb1 = sb.tile([1, 1], F32, tag="zb1")
            nc.vector.tensor_scalar(
                zb1[:], b3, c1[:], np.pi / 4, Alu.mult, Alu.add
            )
            zb2 = sb.tile([1, 1], F32, tag="zb2")
            nc.vector.tensor_scalar(
                zb2[:], c1[:], bias0, 0.0, Alu.subtract, Alu.add
            )
            zb = sb.tile([1, 1], F32, tag="zb")
            nc.vector.tensor_add(zb[:], zb1[:], zb2[:])
            zbc = sb.tile([1, 1], F32, tag="zbc")
            nc.vector.tensor_scalar_add(zbc[:], zb[:], np.pi / 2)
            # amp folded into layer-1 weights
            w1s = sb.tile([1, 32], F32, tag="w1s")
            nc.vector.tensor_scalar_mul(w1s[:], w1[:], amp)

            # ---- coarse coefficient pipeline -------------------------------
            g_sb = sb.tile([1, NCC], F32, tag="g_sb")
            nc.sync.dma_start(g_sb[:], g_d.ap())
            # lfo/amp = cos(arg) = sin(g*step + (step + phi + pi/2))
            cosr = sb.tile([1, NCC], F32, tag="cosr")
            nc.scalar.activation(
                cosr[:], g_sb[:], AF.Sin, bias=sinb[:], scale=step[:]
            )
            # layer 1: [1,NCC] -> [32,NCC]
            ps1 = ps.tile([32, NCC], F32, tag="ps1")
            for c0 in range(0, NCC, MMN):
                n = min(MMN, NCC - c0)
                nc.tensor.matmul(
                    ps1[:, c0 : c0 + n], w1s[:], cosr[0:1, c0 : c0 + n],
                    start=True, stop=True,
                )
            h1 = sb.tile([32, NCC], F32, tag="h1")
            nc.scalar.activation(h1[:], ps1[:], AF.Tanh, bias=b1[:])
            # layer 2: [32,NCC] -> [32,NCC]
            ps2 = ps.tile([32, NCC], F32, tag="ps2")
            for c0 in range(0, NCC, MMN):
                n = min(MMN, NCC - c0)
                nc.tensor.matmul(
                    ps2[:, c0 : c0 + n], w2[:], h1[:, c0 : c0 + n],
                    start=True, stop=True,
                )
            h2 = sb.tile([32, NCC], F32, tag="h2")
            nc.scalar.activation(h2[:], ps2[:], AF.Tanh, bias=b2[:])
            # layer 3: [32,NCC] -> [1,NCC]  (m before +b3; b3 folded into zb)
            ps3 = ps.tile([1, NCC], F32, tag="ps3")
            for c0 in range(0, NCC, MMN):
                n = min(MMN, NCC - c0)
                nc.tensor.matmul(
                    ps3[:, c0 : c0 + n], w3[:], h2[:, c0 : c0 + n],
                    start=True, stop=True,
                )
            # z = pi/4 - d = c1*m + zb ; tan(z) = sin(z)/cos(z)
            sinz = sb.tile([1, NCC], F32, tag="sinz")
            nc.scalar.activation(
                sinz[:], ps3[:], AF.Sin, bias=zb[:], scale=c1[:]
            )
            cosz = sb.tile([1, NCC], F32, tag="cosz")
            nc.scalar.activation(
                cosz[:], ps3[:], AF.Sin, bias=zbc[:], scale=c1[:]
            )
            rc = sb.tile([1, NCC], F32, tag="rc")
            nc.vector.reciprocal(rc[:], cosz[:])
            td = sb.tile([1, NCC], F32, tag="td")
            nc.vector.tensor_mul(td[:], sinz[:], rc[:])
            # a = -p = tanh(-tan(z))
            ac = sb.tile([1, NCC], F32, tag="ac")
            nc.scalar.activation(ac[:], td[:], AF.Tanh, scale=-1.0)

            # ---- redistribute coarse row into per-partition windows --------
            # act[p, i] = ac[4p + i], i in [0, 6)  (stride-4 overlapping)
            act = sb.tile([P, NI + 1], F32, tag="act")
            nc.sync.dma_start(
                act[:], _ap(ac[:], [[1, 1], [L // S, P], [1, NI + 1]])
            )
            dlt = sb.tile([P, NI], F32, tag="dlt")
            nc.vector.tensor_sub(dlt[:], act[:, 1 : NI + 1], act[:, 0:NI])

            # ---- upsample: a_ov = act_rep + dlt_rep * frac -----------------
            a_ov = sb.tile([P, ROW], F32, tag="a_ov")
            a3 = a_ov[:].rearrange("p (c s) -> p c s", s=S)
            dlt_b = dlt[:].unsqueeze(2).broadcast_to((P, NI, S))
            act_b = act[:, 0:NI].unsqueeze(2).broadcast_to((P, NI, S))
            f3 = frac[:].rearrange("p (c s) -> p c s", s=S)
            nc.vector.tensor_tensor(a3, f3, dlt_b, Alu.mult)
            nc.vector.tensor_tensor(a3, a3, act_b, Alu.add)

            # ---- 4 cascaded all-pass stages via native scan ----------------
            s_cur = x_ov
            for k in range(4):
                tmp = sb.tile([P, ROW], F32, tag=f"tmp{k}")
                nc.vector.tensor_mul(tmp[:], a_ov[:], s_cur[:])
                # b[:,1:] = s[:, :-1] - (a*s)[:, 1:]  (col 0 garbage; decays)
                nc.vector.tensor_tensor(
                    tmp[:, 1:ROW], s_cur[:, 0 : ROW - 1], tmp[:, 1:ROW],
                    Alu.subtract,
                )
                y = sb.tile([P, ROW], F32, tag=f"y{k}")
                nc.vector.tensor_tensor_scan(
                    y[:], a_ov[:], tmp[:], 0.0, Alu.mult, Alu.add
                )
                s_cur = y

            # ---- dry/wet mix + store ---------------------------------------
            mix = sb.tile([P, L], F32, tag="mix")
            nc.vector.tensor_add(mix[:], x_ov[:, W:ROW], s_cur[:, W:ROW])
            osb = sb.tile([P, L], F32, tag="osb")
            nc.scalar.activation(osb[:], mix[:], AF.Copy, scale=0.5)
            nc.sync.dma_start(_ap(o_d.ap(), [[L, P], [1, L]]), osb[:])

    nc.compile()
    return nc


def make_in_maps(x, lfo_rate, lfo_stereo_phase_offset, amp, bias, depth,
                 W1, b1, W2, b2, W3, b3):
    x = np.asarray(x, np.float32).reshape(-1)
    frac = np.broadcast_to(
        ((np.arange(ROW) % S) / S).astype(np.float32).reshape(1, ROW), (P, ROW)
    ).copy()
    base = {
        "frac": frac,
        "W1": np.asarray(W1, np.float32).reshape(1, 32),
        "b1": np.asarray(b1, np.float32).reshape(32, 1),
        "W2": np.asarray(W2, np.float32).reshape(32, 32),
        "b2": np.asarray(b2, np.float32).reshape(32, 1),
        "W3": np.asarray(W3, np.float32).reshape(32, 1),
    }
    in_maps = []
    for core in range(NCORES):
        ch, q = divmod(core, 4)
        T0 = QT * q
        if T0 - W >= 0:
            x_ext = x[T0 - W : T0 + QT]
        else:
            x_ext = np.concatenate([np.zeros(W, np.float32), x[0 : T0 + QT]])
        g = ((np.arange(NCC, dtype=np.float64) + (T0 // S) - 1) * S).astype(
            np.float32
        )
        phi = np.float32(0.0 if ch == 0 else np.asarray(
            lfo_stereo_phase_offset, np.float32).reshape(-1)[0])
        scal = np.array(
            [
                [
                    np.float32(np.asarray(lfo_rate).reshape(-1)[0]),
                    phi,
                    np.float32(np.asarray(amp)),
                    np.float32(np.asarray(bias)),
                    np.float32(np.asarray(depth)),
                    np.float32(np.asarray(b3).reshape(-1)[0]),
                    0.0,
                    0.0,
                ]
            ],
            np.float32,
        )
        in_maps.append(
            {
                **base,
                "x_ext": x_ext.reshape(1, W + QT).copy(),
                "g_row": g.reshape(1, NCC),
                "scal": scal,
            }
        )
    return in_maps


_prog_cache = {}


def kernel(**inputs) -> np.ndarray:
    if "nc" not in _prog_cache:
        _prog_cache["nc"] = build_program()
    nc = _prog_cache["nc"]
    in_maps = make_in_maps(**inputs)
    res = bass_utils.run_bass_kernel_spmd(
        nc, in_maps, core_ids=list(range(NCORES))
    )
    out = np.empty((2, T), np.float32)
    for core in range(NCORES):
        ch, q = divmod(core, 4)
        out[ch, QT * q : QT * (q + 1)] = res.results[core]["out"][0]
    return out


# revision 7
# speedup vs baseline: 1.2175x; 1.2175x over previous
"""Trainium2 Bass kernel for nn_PhaserModel: time-varying 4-stage all-pass
phaser driven by an MLP-shaped LFO.

Strategy (8 NeuronCores = 2 stereo channels x 4 time-quarters):
  - The per-sample all-pass coefficient p[t] is a smooth function of the LFO
    phase (rate < 1 Hz).  Each core evaluates cos + the 1x32x32x1 tanh MLP on
    device at a coarse grid (one point per S=128 samples), maps the MLP
    output m through a = -tanh(tan(pi/4 - d(m))) using a degree-6 polynomial
    (host-fitted over the certified m-range; fp32 floor ~2e-8), and linearly
    interpolates a to per-sample resolution on device.
  - The 4 cascaded all-pass stages y[n] = p[n]*s[n] + s[n-1] - p[n]*y[n-1]
    are first-order linear recurrences y[n] = a[n]*y[n-1] + b[n] with
    b[n] = s[n-1] - a[n]*s[n]; they run on the DVE's native
    tensor_tensor_scan.  Each of the 128 partitions scans its own contiguous
    512-sample chunk plus a 128-sample warmup overlap with zero initial
    state: |p| stays well below 1 (~0.27 here), so the warmup error decays
    below 1e-70 and no cross-partition/cross-core carries are needed.
"""

import numpy as np

import concourse.bass as bass
import concourse.bacc as bacc
import concourse.mybir as mybir
import concourse.tile as tile
from concourse import bass_utils

SR = 44100.0
T = 262144
NCORES = 8
QT = T // 4          # output samples per core (time quarter)
P = 128              # SBUF partitions
L = QT // P          # own samples per partition = 512
W = 128              # warmup samples per partition
ROW = W + L          # scanned row length = 640
S = 128              # coarse grid spacing (samples)
NI = ROW // S        # coarse intervals per row = 5
NCC = QT // S + 2    # coarse points per core = 514
DEG = 6              # composite polynomial degree
F32 = mybir.dt.float32
MMN = 512            # max matmul free dim


def _ap(t_ap, pattern, extra_offset=0):
    """Custom [step,count] access pattern on an existing AP's tensor."""
    return bass.AP(t_ap.tensor, t_ap.offset + extra_offset, pattern)


def build_program():
    Alu = mybir.AluOpType
    AF = mybir.ActivationFunctionType

    nc = bacc.Bacc(
        "TRN2", target_bir_lowering=False, debug=False, num_devices=NCORES
    )
    x_d = nc.dram_tensor("x_ext", [1, W + QT], F32, kind="ExternalInput")
    g_d = nc.dram_tensor("g_row", [1, NCC], F32, kind="ExternalInput")
    f_d = nc.dram_tensor("frac", [P, ROW], F32, kind="ExternalInput")
    # wpack[0:32, 0:32]=W2, [:,32]=b2, [:,33]=W3, [:,34]=b1, [32, 0:32]=W1
    wp_d = nc.dram_tensor("wpack", [33, 36], F32, kind="ExternalInput")
    # cpack (row-broadcast): 0=rate 1=phi 2=amp 3=mc 4=inv_mh 5..5+DEG=coeffs
    cp_d = nc.dram_tensor("cpack", [P, 16], F32, kind="ExternalInput")
    o_d = nc.dram_tensor("out", [1, QT], F32, kind="ExternalOutput")

    with tile.TileContext(nc) as tc:
        with (
            tc.tile_pool(name="sb", bufs=1) as sb,
            tc.tile_pool(name="ps", bufs=1, space=bass.MemorySpace.PSUM) as ps,
        ):
            # ---- input DMAs -------------------------------------------------
            wp = sb.tile([33, 36], F32, tag="wp")
            nc.sync.dma_start(wp[:], wp_d.ap())
            cp = sb.tile([P, 16], F32, tag="cp")
            nc.sync.dma_start(cp[:], cp_d.ap())
            g_sb = sb.tile([1, NCC], F32, tag="g_sb")
            nc.sync.dma_start(g_sb[:], g_d.ap())
            x_ov = sb.tile([P, ROW], F32, tag="x_ov")
            nc.sync.dma_start(x_ov[:], _ap(x_d.ap(), [[L, P], [1, ROW]]))
            frac = sb.tile([P, ROW], F32, tag="frac")
            nc.sync.dma_start(frac[:], f_d.ap())

            W2ap = wp[0:32, 0:32]
            b2ap = wp[0:32, 32:33]
            W3ap = wp[0:32, 33:34]
            b1ap = wp[0:32, 34:35]
            W1ap = wp[32:33, 0:32]
            rate = cp[0:1, 0:1]
            phi = cp[0:1, 1:2]
            amp = cp[0:1, 2:3]

            # ---- tiny scalar prep ------------------------------------------
            step = sb.tile([1, 1], F32, tag="step")
            nc.vector.tensor_scalar_mul(step[:], rate, 2.0 * np.pi / SR)
            sinb = sb.tile([1, 1], F32, tag="sinb")
            nc.vector.tensor_scalar(
                sinb[:], step[:], phi, np.pi / 2, Alu.add, Alu.add
            )
            w1s = sb.tile([1, 32], F32, tag="w1s")
            nc.vector.tensor_scalar_mul(w1s[:], W1ap, amp)

            # x_half precompute (off critical path, frees the tail)
            xh = sb.tile([P, L], F32, tag="xh")
            nc.vector.tensor_scalar_mul(xh[:], x_ov[:, W:ROW], 0.5)

            # ---- coarse pipeline: cos -> MLP -> m ---------------------------
            # lfo/amp = cos(arg) = sin(g*step + (step + phi + pi/2))
            cosr = sb.tile([1, NCC], F32, tag="cosr")
            nc.scalar.activation(
                cosr[:], g_sb[:], AF.Sin, bias=sinb[:], scale=step[:]
            )
            ps1 = ps.tile([32, NCC], F32, tag="ps1")
            for c0 in range(0, NCC, MMN):
                n = min(MMN, NCC - c0)
                nc.tensor.matmul(
                    ps1[:, c0 : c0 + n], w1s[:], cosr[0:1, c0 : c0 + n],
                    start=True, stop=True,
                )
            h1 = sb.tile([32, NCC], F32, tag="h1")
            nc.scalar.activation(h1[:], ps1[:], AF.Tanh, bias=b1ap)
            ps2 = ps.tile([32, NCC], F32, tag="ps2")
            for c0 in range(0, NCC, MMN):
                n = min(MMN, NCC - c0)
                nc.tensor.matmul(
                    ps2[:, c0 : c0 + n], W2ap, h1[:, c0 : c0 + n],
                    start=True, stop=True,
                )
            h2 = sb.tile([32, NCC], F32, tag="h2")
            nc.scalar.activation(h2[:], ps2[:], AF.Tanh, bias=b2ap)
            ps3 = ps.tile([1, NCC], F32, tag="ps3")
            for c0 in range(0, NCC, MMN):
                n = min(MMN, NCC - c0)
                nc.tensor.matmul(
                    ps3[:, c0 : c0 + n], W3ap, h2[:, c0 : c0 + n],
                    start=True, stop=True,
                )

            # ---- redistribute m into per-partition windows ------------------
            # m_t[p, i] = m[4p + i], i in [0, 6)
            m_row = sb.tile([1, NCC], F32, tag="m_row")
            nc.scalar.activation(m_row[:], ps3[:], AF.Copy)
            m_t = sb.tile([P, NI + 1], F32, tag="m_t")
            nc.sync.dma_start(
                m_t[:], _ap(m_row[:], [[1, 1], [L // S, P], [1, NI + 1]])
            )

            # ---- composite poly: a = -tanh(tan(pi/4 - d(m))) ---------------
            # xs = (m - mc) * inv_mh ; a = polyval(cf, xs)
            xs = sb.tile([P, NI + 1], F32, tag="xs")
            nc.vector.tensor_scalar(
                xs[:], m_t[:], cp[:, 3:4], cp[:, 4:5], Alu.subtract, Alu.mult
            )
            acc = sb.tile([P, NI + 1], F32, tag="acc")
            nc.vector.tensor_scalar_mul(acc[:], xs[:], cp[:, 5:6])
            for k in range(1, DEG):
                nc.vector.scalar_tensor_tensor(
                    acc[:], acc[:], cp[:, 5 + k : 6 + k], xs[:],
                    Alu.add, Alu.mult,
                )
            a_t = sb.tile([P, NI + 1], F32, tag="a_t")
            nc.vector.tensor_scalar(
                a_t[:], acc[:], cp[:, 5 + DEG : 6 + DEG], None, Alu.add
            )

            # ---- upsample: a_ov = a_rep + dlt_rep * frac --------------------
            dlt = sb.tile([P, NI], F32, tag="dlt")
            nc.vector.tensor_sub(dlt[:], a_t[:, 1 : NI + 1], a_t[:, 0:NI])
            a_ov = sb.tile([P, ROW], F32, tag="a_ov")
            a3 = a_ov[:].rearrange("p (c s) -> p c s", s=S)
            dlt_b = dlt[:].unsqueeze(2).broadcast_to((P, NI, S))
            act_b = a_t[:, 0:NI].unsqueeze(2).broadcast_to((P, NI, S))
            f3 = frac[:].rearrange("p (c s) -> p c s", s=S)
            nc.vector.tensor_tensor(a3, f3, dlt_b, Alu.mult)
            nc.vector.tensor_tensor(a3, a3, act_b, Alu.add)

            # ---- 4 cascaded all-pass stages via native scan ----------------
            s_cur = x_ov
            for k in range(4):
                tmp = sb.tile([P, ROW], F32, tag=f"tmp{k}")
                nc.vector.tensor_mul(tmp[:], a_ov[:], s_cur[:])
                # b[:,1:] = s[:, :-1] - (a*s)[:, 1:]  (col 0 garbage; decays)
                nc.vector.tensor_tensor(
                    tmp[:, 1:ROW], s_cur[:, 0 : ROW - 1], tmp[:, 1:ROW],
                    Alu.subtract,
                )
                y = sb.tile([P, ROW], F32, tag=f"y{k}")
                nc.vector.tensor_tensor_scan(
                    y[:], a_ov[:], tmp[:], 0.0, Alu.mult, Alu.add
                )
                s_cur = y

            # ---- dry/wet mix + store: out = 0.5*y4 + xh --------------------
            osb = sb.tile([P, L], F32, tag="osb")
            nc.vector.scalar_tensor_tensor(
                osb[:], s_cur[:, W:ROW], 0.5, xh[:], Alu.mult, Alu.add
            )
            nc.sync.dma_start(_ap(o_d.ap(), [[L, P], [1, L]]), osb[:])

    nc.compile()
    return nc


def _fit_composite(lfo_rate, off, amp, bias, depth, W1, b1, W2, b2, W3, b3):
    """Host-side: certify the m-range via a coarse probe and fit the
    degree-DEG polynomial for a(m) = -tanh(tan(pi/4 - d(m))) in the scaled
    variable xs = (m - mc)/mh.  Only O(1k) scalar work independent of T."""
    W1, b1, W2, b2, W3, b3 = [
        np.asarray(v, np.float64) for v in (W1, b1, W2, b2, W3, b3)
    ]
    rate = float(np.asarray(lfo_rate).reshape(-1)[0])
    amp, bias, depth = (float(np.asarray(v)) for v in (amp, bias, depth))
    b3v = float(b3.reshape(-1)[0])
    c1 = -depth / 2.0
    zb = np.pi / 4 - bias - depth / 2.0 + c1 * b3v
    step = 2.0 * np.pi * rate / SR
    n = np.linspace(0.0, T, 1025)
    ms = []
    for phi in (0.0, float(np.asarray(off).reshape(-1)[0])):
        lfo = amp * np.cos((n + 1.0) * step + phi)
        h = np.tanh(lfo[:, None] @ W1.reshape(1, 32) + b1.reshape(32))
        h = np.tanh(h @ W2 + b2.reshape(32))
        ms.append((h @ W3.reshape(32, 1))[:, 0])
    ms = np.concatenate(ms)
    pad = 0.3 + 0.1 * (ms.max() - ms.min())
    mlo, mhi = ms.min() - pad, ms.max() + pad
    mc, mh = 0.5 * (mlo + mhi), 0.5 * (mhi - mlo)
    wlo, whi = sorted((c1 * mlo + zb, c1 * mhi + zb))
    assert -1.55 < wlo and whi < 1.55, f"tan arg out of range: {wlo},{whi}"
    m = np.linspace(mlo, mhi, 4001)
    xsp = (m - mc) / mh
    a_true = -np.tanh(np.tan(c1 * m + zb))
    cf = np.polyfit(xsp, a_true, DEG)
    fit_err = np.abs(np.polyval(cf, xsp) - a_true).max()
    assert fit_err < 1e-6, f"poly fit error too large: {fit_err}"
    return mc, mh, cf


def make_in_maps(x, lfo_rate, lfo_stereo_phase_offset, amp, bias, depth,
                 W1, b1, W2, b2, W3, b3):
    x = np.asarray(x, np.float32).reshape(-1)
    off = np.asarray(lfo_stereo_phase_offset, np.float32).reshape(-1)[0]
    mc, mh, cf = _fit_composite(
        lfo_rate, off, amp, bias, depth, W1, b1, W2, b2, W3, b3
    )
    frac = np.broadcast_to(
        ((np.arange(ROW) % S) / S).astype(np.float32).reshape(1, ROW), (P, ROW)
    ).copy()
    wpack = np.zeros((33, 36), np.float32)
    wpack[0:32, 0:32] = np.asarray(W2, np.float32)
    wpack[0:32, 32] = np.asarray(b2, np.float32).reshape(32)
    wpack[0:32, 33] = np.asarray(W3, np.float32).reshape(32)
    wpack[0:32, 34] = np.asarray(b1, np.float32).reshape(32)
    wpack[32, 0:32] = np.asarray(W1, np.float32).reshape(32)
    base = {"frac": frac, "wpack": wpack}
    in_maps = []
    for core in range(NCORES):
        ch, q = divmod(core, 4)
        T0 = QT * q
        if T0 - W >= 0:
            x_ext = x[T0 - W : T0 + QT]
        else:
            x_ext = np.concatenate([np.zeros(W, np.float32), x[0 : T0 + QT]])
        g = ((np.arange(NCC, dtype=np.float64) + (T0 // S) - 1) * S).astype(
            np.float32
        )
        crow = np.zeros(16, np.float32)
        crow[0] = np.float32(np.asarray(lfo_rate).reshape(-1)[0])
        crow[1] = np.float32(0.0 if ch == 0 else off)
        crow[2] = np.float32(np.asarray(amp))
        crow[3] = np.float32(mc)
        crow[4] = np.float32(1.0 / mh)
        crow[5 : 6 + DEG] = cf.astype(np.float32)
        cpack = np.broadcast_to(crow.reshape(1, 16), (P, 16)).copy()
        in_maps.append(
            {
                **base,
                "x_ext": x_ext.reshape(1, W + QT).copy(),
                "g_row": g.reshape(1, NCC),
                "cpack": cpack,
            }
        )
    return in_maps


_prog_cache = {}


def kernel(**inputs) -> np.ndarray:
    if "nc" not in _prog_cache:
        _prog_cache["nc"] = build_program()
    nc = _prog_cache["nc"]
    in_maps = make_in_maps(**inputs)
    res = bass_utils.run_bass_kernel_spmd(
        nc, in_maps, core_ids=list(range(NCORES))
    )
    out = np.empty((2, T), np.float32)
    for core in range(NCORES):
        ch, q = divmod(core, 4)
        out[ch, QT * q : QT * (q + 1)] = res.results[core]["out"][0]
    return out


# revision 12
# speedup vs baseline: 1.2381x; 1.0169x over previous
"""Trainium2 Bass kernel for nn_PhaserModel: time-varying 4-stage all-pass
phaser driven by an MLP-shaped LFO.

Strategy (8 NeuronCores = 2 stereo channels x 4 time-quarters):
  - The per-sample all-pass coefficient p[t] is a smooth function of the LFO
    phase (rate < 1 Hz).  Each core evaluates cos + the 1x32x32x1 tanh MLP on
    device at a coarse grid (one point per S=128 samples), maps the MLP
    output m through a = -tanh(tan(pi/4 - d(m))) using a degree-4 polynomial
    (host-fitted over a certified padded m-range; fp32 error ~3e-7), and
    linearly interpolates a to per-sample resolution on device.
  - The 4 cascaded all-pass stages y[n] = p[n]*s[n] + s[n-1] - p[n]*y[n-1]
    are first-order linear recurrences y[n] = a[n]*y[n-1] + b[n] with
    b[n] = s[n-1] - a[n]*s[n]; they run on the DVE's native
    tensor_tensor_scan.  Each of the 128 partitions scans its own contiguous
    512-sample chunk plus a 128-sample warmup overlap with zero initial
    state: |p| stays well below 1 (~0.27 here), so the warmup error decays
    below 1e-70 and no cross-partition/cross-core carries are needed.
"""

import numpy as np

import concourse.bass as bass
import concourse.bacc as bacc
import concourse.mybir as mybir
import concourse.tile as tile
from concourse import bass_utils

SR = 44100.0
T = 262144
NCORES = 8
QT = T // 4          # output samples per core (time quarter)
P = 128              # SBUF partitions
L = QT // P          # own samples per partition = 512
W = 256              # warmup samples per partition
ROW = W + L          # scanned row length = 768
S = 256              # coarse grid spacing (samples)
NI = ROW // S        # coarse intervals per row = 3
NCC = QT // S + 2    # coarse points per core = 258
DEG = 4              # composite polynomial degree
F32 = mybir.dt.float32
F32R = mybir.dt.float32r
MMN = 512            # max matmul free dim

# packed-constant column layout
C_FRAC = 0                      # [P, 0:640]   frac
C_SC = ROW                      # [P, 640:656] scalars+coeffs (row-broadcast)
C_WP = C_SC + 16                # [0:33, 656:692] weights
C_G = C_WP + 36                 # [0:1, 692:1206] coarse grid
CW_TOT = C_G + NCC              # 1206


def _ap(t_ap, pattern, extra_offset=0):
    """Custom [step,count] access pattern on an existing AP's tensor."""
    return bass.AP(t_ap.tensor, t_ap.offset + extra_offset, pattern)


def build_program():
    Alu = mybir.AluOpType
    AF = mybir.ActivationFunctionType

    nc = bacc.Bacc(
        "TRN2", target_bir_lowering=False, debug=False, num_devices=NCORES
    )
    x_d = nc.dram_tensor("x_ext", [1, W + QT], F32, kind="ExternalInput")
    k_d = nc.dram_tensor("kpack", [P, CW_TOT], F32, kind="ExternalInput")
    o_d = nc.dram_tensor("out", [1, QT], F32, kind="ExternalOutput")

    with tile.TileContext(nc) as tc:
        with (
            tc.tile_pool(name="sb", bufs=1) as sb,
            tc.tile_pool(name="ps", bufs=1, space=bass.MemorySpace.PSUM) as ps,
        ):
            # ---- input DMAs (constants on sync, x on gpsimd queue) ---------
            kp = sb.tile([P, CW_TOT], F32, tag="kp")
            nc.sync.dma_start(kp[:], k_d.ap())
            x_ov = sb.tile([P, ROW], F32, tag="x_ov")
            nc.gpsimd.dma_start(x_ov[:], _ap(x_d.ap(), [[L, P], [1, ROW]]))

            frac = kp[:, 0:ROW]
            rate = kp[0:1, C_SC : C_SC + 1]
            phi = kp[0:1, C_SC + 1 : C_SC + 2]
            amp = kp[0:1, C_SC + 2 : C_SC + 3]

            def cf_ap(k):  # poly coeff cf[k] (highest-first), [128,1]
                return kp[:, C_SC + 3 + k : C_SC + 4 + k]

            W2ap = kp[0:32, C_WP : C_WP + 32]
            b2ap = kp[0:32, C_WP + 32 : C_WP + 33]
            W3ap = kp[0:32, C_WP + 33 : C_WP + 34]
            b1ap = kp[0:32, C_WP + 34 : C_WP + 35]
            W1ap = kp[32:33, C_WP : C_WP + 32]
            g_ap = kp[0:1, C_G : C_G + NCC]

            # ---- tiny scalar prep ------------------------------------------
            step = sb.tile([1, 1], F32, tag="step")
            nc.vector.tensor_scalar_mul(step[:], rate, 2.0 * np.pi / SR)
            sinb = sb.tile([1, 1], F32, tag="sinb")
            nc.vector.tensor_scalar(
                sinb[:], step[:], phi, np.pi / 2, Alu.add, Alu.add
            )
            w1s = sb.tile([1, 32], F32, tag="w1s")
            nc.vector.tensor_scalar_mul(w1s[:], W1ap, amp)

            # x_half precompute (off critical path, frees the tail)
            xh = sb.tile([P, L], F32, tag="xh")
            nc.vector.tensor_scalar_mul(xh[:], x_ov[:, W:ROW], 0.5)

            # ---- coarse pipeline: cos -> MLP -> m ---------------------------
            # lfo/amp = cos(arg) = sin(g*step + (step + phi + pi/2))
            cosr = sb.tile([1, NCC], F32, tag="cosr")
            nc.scalar.activation(
                cosr[:], g_ap, AF.Sin, bias=sinb[:], scale=step[:]
            )
            ps1 = ps.tile([32, NCC], F32, tag="ps1")
            for c0 in range(0, NCC, MMN):
                n = min(MMN, NCC - c0)
                nc.tensor.matmul(
                    ps1[:, c0 : c0 + n], w1s[:], cosr[0:1, c0 : c0 + n],
                    start=True, stop=True,
                )
            h1 = sb.tile([32, NCC], F32, tag="h1")
            nc.scalar.activation(h1[:], ps1[:], AF.Tanh, bias=b1ap)
            ps2 = ps.tile([32, NCC], F32, tag="ps2")
            for c0 in range(0, NCC, MMN):
                n = min(MMN, NCC - c0)
                nc.tensor.matmul(
                    ps2[:, c0 : c0 + n], W2ap, h1[:, c0 : c0 + n],
                    start=True, stop=True,
                )
            h2 = sb.tile([32, NCC], F32, tag="h2")
            nc.scalar.activation(h2[:], ps2[:], AF.Tanh, bias=b2ap)
            ps3 = ps.tile([1, NCC], F32, tag="ps3")
            for c0 in range(0, NCC, MMN):
                n = min(MMN, NCC - c0)
                nc.tensor.matmul(
                    ps3[:, c0 : c0 + n], W3ap, h2[:, c0 : c0 + n],
                    start=True, stop=True,
                )

            # ---- redistribute m into per-partition windows ------------------
            # m_t[p, i] = m[4p + i], i in [0, 6)
            m_row = sb.tile([1, NCC], F32, tag="m_row")
            nc.scalar.activation(m_row[:], ps3[:], AF.Copy)
            m_t = sb.tile([P, NI + 1], F32, tag="m_t")
            nc.gpsimd.dma_start(
                m_t[:], _ap(m_row[:], [[1, 1], [L // S, P], [1, NI + 1]])
            )

            # ---- composite poly a(m), Horner sans constant term ------------
            acc = sb.tile([P, NI + 1], F32, tag="acc")
            nc.vector.tensor_scalar_mul(acc[:], m_t[:], cf_ap(0))
            for k in range(1, DEG):
                nc.vector.scalar_tensor_tensor(
                    acc[:], acc[:], cf_ap(k), m_t[:], Alu.add, Alu.mult
                )
            # acc = a(m) - cf[DEG]; the constant folds into the upsample add

            # ---- upsample: a_ov = (acc_rep + cf[DEG]) + dlt_rep * frac -----
            dlt = sb.tile([P, NI], F32, tag="dlt")
            nc.vector.tensor_sub(dlt[:], acc[:, 1 : NI + 1], acc[:, 0:NI])
            a_ov = sb.tile([P, ROW], F32, tag="a_ov")
            a3 = a_ov[:].rearrange("p (c s) -> p c s", s=S)
            dlt_b = dlt[:].unsqueeze(2).broadcast_to((P, NI, S))
            acc_b = acc[:, 0:NI].unsqueeze(2).broadcast_to((P, NI, S))
            f3 = frac.rearrange("p (c s) -> p c s", s=S)
            nc.vector.tensor_tensor(a3, f3, dlt_b, Alu.mult)
            nc.vector.scalar_tensor_tensor(
                a3, acc_b, cf_ap(DEG), a3, Alu.add, Alu.add
            )

            # ---- 4 cascaded all-pass stages via native scan ----------------
            s_cur = x_ov
            for k in range(4):
                tmp = sb.tile([P, ROW], F32, tag=f"tmp{k}")
                nc.vector.tensor_mul(tmp[:], a_ov[:], s_cur[:])
                # b[:,1:] = s[:, :-1] - (a*s)[:, 1:]  (col 0 garbage; decays)
                nc.vector.tensor_tensor(
                    tmp[:, 1:ROW], s_cur[:, 0 : ROW - 1], tmp[:, 1:ROW],
                    Alu.subtract,
                )
                y = sb.tile([P, ROW], F32, tag=f"y{k}")
                nc.vector.tensor_tensor_scan(
                    y[:], a_ov[:], tmp[:], 0.0, Alu.mult, Alu.add
                )
                s_cur = y

            # ---- dry/wet mix + store: out = 0.5*y4 + xh --------------------
            osb = sb.tile([P, L], F32, tag="osb")
            nc.vector.scalar_tensor_tensor(
                osb[:], s_cur[:, W:ROW], 0.5, xh[:], Alu.mult, Alu.add
            )
            nc.sync.dma_start(_ap(o_d.ap(), [[L, P], [1, L]]), osb[:])

    nc.compile()
    return nc


def _fit_composite(lfo_rate, off, amp, bias, depth, W1, b1, W2, b2, W3, b3):
    """Host-side: certify the m-range via a coarse probe and fit the
    degree-DEG polynomial for a(m) = -tanh(tan(pi/4 - d(m))) in raw m.
    Only O(1k) scalar work independent of T."""
    W1, b1, W2, b2, W3, b3 = [
        np.asarray(v, np.float64) for v in (W1, b1, W2, b2, W3, b3)
    ]
    rate = float(np.asarray(lfo_rate).reshape(-1)[0])
    amp, bias, depth = (float(np.asarray(v)) for v in (amp, bias, depth))
    b3v = float(b3.reshape(-1)[0])
    c1 = -depth / 2.0
    zb = np.pi / 4 - bias - depth / 2.0 + c1 * b3v
    step = 2.0 * np.pi * rate / SR
    n = np.linspace(0.0, T, 1025)
    ms = []
    for phi in (0.0, float(np.asarray(off).reshape(-1)[0])):
        lfo = amp * np.cos((n + 1.0) * step + phi)
        h = np.tanh(lfo[:, None] @ W1.reshape(1, 32) + b1.reshape(32))
        h = np.tanh(h @ W2 + b2.reshape(32))
        ms.append((h @ W3.reshape(32, 1))[:, 0])
    ms = np.concatenate(ms)
    pad = 0.3 + 0.1 * (ms.max() - ms.min())
    mlo, mhi = ms.min() - pad, ms.max() + pad
    wlo, whi = sorted((c1 * mlo + zb, c1 * mhi + zb))
    assert -1.55 < wlo and whi < 1.55, f"tan arg out of range: {wlo},{whi}"
    m = np.linspace(mlo, mhi, 4001)
    a_true = -np.tanh(np.tan(c1 * m + zb))
    cf = np.polyfit(m, a_true, DEG)
    fit_err = np.abs(np.polyval(cf, m) - a_true).max()
    assert fit_err < 5e-6, f"poly fit error too large: {fit_err}"
    return cf


def make_in_maps(x, lfo_rate, lfo_stereo_phase_offset, amp, bias, depth,
                 W1, b1, W2, b2, W3, b3):
    x = np.asarray(x, np.float32).reshape(-1)
    off = np.asarray(lfo_stereo_phase_offset, np.float32).reshape(-1)[0]
    cf = _fit_composite(
        lfo_rate, off, amp, bias, depth, W1, b1, W2, b2, W3, b3
    )
    in_maps = []
    for core in range(NCORES):
        ch, q = divmod(core, 4)
        T0 = QT * q
        if T0 - W >= 0:
            x_ext = x[T0 - W : T0 + QT]
        else:
            x_ext = np.concatenate([np.zeros(W, np.float32), x[0 : T0 + QT]])
        kp = np.zeros((P, CW_TOT), np.float32)
        kp[:, 0:ROW] = ((np.arange(ROW) % S) / S).astype(np.float32)
        kp[:, C_SC + 0] = np.float32(np.asarray(lfo_rate).reshape(-1)[0])
        kp[:, C_SC + 1] = np.float32(0.0 if ch == 0 else off)
        kp[:, C_SC + 2] = np.float32(np.asarray(amp))
        for k in range(DEG + 1):
            kp[:, C_SC + 3 + k] = np.float32(cf[k])
        kp[0:32, C_WP : C_WP + 32] = np.asarray(W2, np.float32)
        kp[0:32, C_WP + 32] = np.asarray(b2, np.float32).reshape(32)
        kp[0:32, C_WP + 33] = np.asarray(W3, np.float32).reshape(32)
        kp[0:32, C_WP + 34] = np.asarray(b1, np.float32).reshape(32)
        kp[32, C_WP : C_WP + 32] = np.asarray(W1, np.float32).reshape(32)
        kp[0, C_G : C_G + NCC] = (
            (np.arange(NCC, dtype=np.float64) + (T0 // S) - 1) * S
        ).astype(np.float32)
        in_maps.append(
            {"x_ext": x_ext.reshape(1, W + QT).copy(), "kpack": kp}
        )
    return in_maps


_prog_cache = {}


def kernel(**inputs) -> np.ndarray:
    if "nc" not in _prog_cache:
        _prog_cache["nc"] = build_program()
    nc = _prog_cache["nc"]
    in_maps = make_in_maps(**inputs)
    res = bass_utils.run_bass_kernel_spmd(
        nc, in_maps, core_ids=list(range(NCORES))
    )
    out = np.empty((2, T), np.float32)
    for core in range(NCORES):
        ch, q = divmod(core, 4)
        out[ch, QT * q : QT * (q + 1)] = res.results[core]["out"][0]
    return out
